# revision 1
# baseline (speedup 1.0000x reference)
"""CalibrationAttention Trainium2 kernel (bf16 restructure).

Data-parallel over batch across 8 NeuronCores (2 instances per core).
Self-contained: hardcodes shapes from the problem spec.

Changes vs the fp32r baseline (217us):
  - everything bf16 (rel err ~5e-3 vs 2e-2 budget): x and all weights are
    host-cast; matmul rate is identical to fp32r>=256 but transposes are 2x
    cheaper, DMA halves, and free-dim evenness constraints disappear.
  - xT comes from the DMA crossbar transpose (14ns per 16x128 tile, runs on
    the DMA engines) instead of PE transposes + DVE evacuations. x is
    host-padded to 592 rows so the xbar tiling constraint (rows % 16) holds.
  - alpha = head_scale/temperature is folded into the exp activation's
    per-partition scale operand instead of scaling q rows (drops the qkT
    scaling muls and removes alpha from the qk-gen critical path).
  - qkv/proj weights are loaded once (not per instance) and the qk weight
    DMA is chunked per-mo so the first matmul starts ~4us in instead of 22us.
  - temperature MLP in natural orientation: h = cls @ W1 as one [1,384] psum
    row (6 matmuls of F=384) + DVE relu/dot instead of 21 tiny matmuls.
  - proj bias stays on DVE; y evacuated from SBUF as before.
"""

import os
from contextlib import ExitStack

import numpy as np
import ml_dtypes

import concourse.bass as bass
import concourse.tile as tile
from concourse import bacc, mybir
from concourse._compat import with_exitstack

F32 = mybir.dt.float32
BF16 = mybir.dt.bfloat16

B, N, C = 16, 577, 768
H, D, HID = 12, 64, 384
P = 128
KO = C // P            # 6 c-tiles
NT = (N + P - 1) // P  # 5 n-tiles (128,128,128,128,65)
IW = 290               # i-chunk width; i padded to 580 (phantom cols 577:580
NPAD = 2 * IW          # are zero in x so they only produce unread aT columns)
NXP = 592              # host-padded x rows (multiple of 16 for xbar transpose)
ICH = (0, IW)
CCH = ((0, 512), (512, 256))  # chunking for 768-wide matmul outputs
SCALE = D ** -0.5
TMIN, TMAX = 0.5, 3.0
NCORES = 8
BPC = B // NCORES      # 2 instances per core


def _rows(mo):
    return P if mo < NT - 1 else N - (NT - 1) * P  # 65 tail


@with_exitstack
def _emit(ctx: ExitStack, tc: tile.TileContext, io: dict, dbg: dict | None = None):
    nc = tc.nc
    AF = mybir.ActivationFunctionType
    ALU = mybir.AluOpType

    x_d = io["x"]
    qkvw_d = io["qkv_w"]
    projw_d = io["proj_w"]
    projb_d = io["proj_b"]
    tw1_d = io["t_w1"]
    tb1_d = io["t_b1"]
    tw2_d = io["t_w2"]
    tb2_d = io["t_b2"]
    y_d = io["y"]

    const = ctx.enter_context(tc.tile_pool(name="const", bufs=1))
    xT_pool = ctx.enter_context(tc.tile_pool(name="xTp", bufs=1))
    qkT_p = ctx.enter_context(tc.tile_pool(name="qkT", bufs=1))
    v_p = ctx.enter_context(tc.tile_pool(name="v", bufs=1))
    pT_p = ctx.enter_context(tc.tile_pool(name="pT", bufs=1))
    aT_pool = ctx.enter_context(tc.tile_pool(name="aTp", bufs=1))
    y_p = ctx.enter_context(tc.tile_pool(name="y", bufs=2))
    sm_p = ctx.enter_context(tc.tile_pool(name="sm", bufs=2))
    rd_p = ctx.enter_context(tc.tile_pool(name="rd", bufs=1))
    rb_p = ctx.enter_context(tc.tile_pool(name="rb", bufs=1))
    tmp_p = ctx.enter_context(tc.tile_pool(name="tmp", bufs=1))
    ps1 = ctx.enter_context(tc.tile_pool(name="ps1", bufs=2, space="PSUM"))
    ps2 = ctx.enter_context(tc.tile_pool(name="ps2", bufs=2, space="PSUM"))

    # ---- instance-0 x transpose first, then qk weights (chunked per-mo so
    # mo=0 lands fast); instance-1's transpose can wait until the qk/v
    # weights are in flight ----
    xTs = []
    for b in range(BPC):
        xTs.append(xT_pool.tile([P, KO, NXP], BF16, tag=f"xT{b}", name=f"xT_{b}"))
    nc.sync.dma_start(xTs[0][:], x_d[0], transpose=True)
    nc.sync.dma_start(xTs[1][:], x_d[1], transpose=True)

    wqk_sb = const.tile([P, KO, 2 * C], BF16, tag="wqk")
    for mo in range(2 * KO):
        nc.sync.dma_start(
            wqk_sb[:, :, mo * P:(mo + 1) * P],
            qkvw_d[:, mo * P:(mo + 1) * P].rearrange("(ko p) m -> p ko m", p=P))
    wv_sb = const.tile([P, KO, C], BF16, tag="wv")
    nc.sync.dma_start(wv_sb[:], qkvw_d[:, 2 * C:3 * C].rearrange("(ko p) m -> p ko m", p=P))
    projw_sb = const.tile([P, KO, C], BF16, tag="pw")
    nc.sync.dma_start(projw_sb[:], projw_d.rearrange("(ko p) m -> p ko m", p=P))
    tw1_sb = const.tile([P, KO, HID], BF16, tag="tw1")
    nc.sync.dma_start(tw1_sb[:], tw1_d.rearrange("(ko p) m -> p ko m", p=P))
    tw2_sb = const.tile([1, HID], BF16, tag="tw2")
    nc.sync.dma_start(tw2_sb[:], tw2_d)
    tb1_sb = const.tile([1, HID], F32, tag="tb1")
    nc.sync.dma_start(tb1_sb[:], tb1_d.unsqueeze(0))
    b2_sb = const.tile([1, 1], F32, tag="b2")
    nc.sync.dma_start(b2_sb[:], tb2_d.unsqueeze(0))
    nb2_sb = const.tile([1, 1], F32, tag="nb2")
    nc.vector.tensor_scalar_mul(nb2_sb[:], b2_sb[:], -1.0)
    pb_bc = const.tile([P, C], F32, tag="pb")
    nc.sync.dma_start(pb_bc[:], projb_d.unsqueeze(0).to_broadcast([P, C]))

    def phase_M(b, xT):
        """temperature MLP -> alpha_bc [128, 1] (= head_scale / temperature)."""
        hps = ps1.tile([P, 2, 512], F32, tag="ps1", name=f"hps_{b}")
        for ko in range(KO):
            nc.tensor.matmul(hps[0:1, 0, :HID],
                             lhsT=xT[:, ko, 0:1],
                             rhs=tw1_sb[:, ko],
                             start=(ko == 0), stop=(ko == KO - 1))
        hsb = sm_p.tile([1, HID], F32, tag="hsb", name=f"hsb_{b}")
        # h = relu(cls @ w1 + b1)
        nc.vector.tensor_tensor(hsb[:], hps[0:1, 0, :HID], tb1_sb[:], op=ALU.add)
        nc.vector.tensor_scalar_max(hsb[:], hsb[:], 0.0)
        hw2 = sm_p.tile([1, HID], F32, tag="hw2", name=f"hw2_{b}")
        nc.vector.tensor_tensor(hw2[:], hsb[:], tw2_sb[:], op=ALU.mult)
        ssb = sm_p.tile([1, 1], F32, tag="ssb", name=f"ssb_{b}")
        nc.vector.tensor_reduce(ssb[:], hw2[:], axis=mybir.AxisListType.X, op=ALU.add)
        esb = sm_p.tile([1, 1], F32, tag="esb", name=f"esb_{b}")
        # e = exp(-(s + b2)); sigmoid = 1/(1+e)
        nc.scalar.activation(esb[:], ssb[:], AF.Exp, bias=nb2_sb[:], scale=-1.0)
        dsb = sm_p.tile([1, 1], F32, tag="dsb", name=f"dsb_{b}")
        nc.vector.tensor_scalar_add(dsb[:], esb[:], 1.0)
        t2 = sm_p.tile([1, 1], F32, tag="t2", name=f"t2_{b}")
        nc.vector.reciprocal(t2[:], dsb[:])
        usb = sm_p.tile([1, 1], F32, tag="usb", name=f"usb_{b}")
        nc.vector.tensor_scalar(usb[:], t2[:], TMAX - TMIN, TMIN, op0=ALU.mult, op1=ALU.add)
        rsb = sm_p.tile([1, 1], F32, tag="rsb", name=f"rsb_{b}")
        nc.vector.reciprocal(rsb[:], usb[:])
        asb = sm_p.tile([1, 1], F32, tag="asb", name=f"asb_{b}")
        nc.vector.tensor_scalar_mul(asb[:], rsb[:], SCALE)  # alpha = scale / temp
        alpha_bc = sm_p.tile([P, 1], F32, tag="abc", name=f"abc_{b}")
        nc.gpsimd.partition_broadcast(alpha_bc[:], asb[:])
        if dbg is not None and b == 0:
            nc.sync.dma_start(dbg["alpha"], alpha_bc[:])
            nc.sync.dma_start(dbg["xT"], xT[:, :, :NPAD])
        return alpha_bc

    def gen_QK(b, xT, out):
        """qkT [128, 12, NPAD] bf16 (c3 = mo*128 + p); yields after each mo."""
        qkT = qkT_p.tile([P, 2 * KO, NPAD], BF16, tag=f"qkT{b}", name=f"qkT_{b}")
        out["qkT"] = qkT
        for mo in range(2 * KO):
            for ci, i0 in enumerate(ICH):
                pq = ps1.tile([P, 2, 512], F32, tag="ps1", name=f"pq_{b}_{mo}_{i0}")
                for ko in range(KO):
                    nc.tensor.matmul(pq[:, ci, :IW],
                                     lhsT=wqk_sb[:, ko, mo * P:(mo + 1) * P],
                                     rhs=xT[:, ko, i0:i0 + IW],
                                     start=(ko == 0), stop=(ko == KO - 1))
                nc.vector.tensor_copy(qkT[:, mo, i0:i0 + IW], pq[:, ci, :IW])
            yield
        if dbg is not None and b == 0:
            nc.sync.dma_start(dbg["qkT"], qkT[:])

    def gen_V(b, xT, out):
        """v_sb [128, NT, H*66] bf16: col 64 of each head = ones (denominator).

        Tail rows beyond n=577 come from zero-padded x columns, so they are
        zero in the matmul output; the ones column is harmless there because
        the matching pT rows stay zero.
        """
        vsb = v_p.tile([P, NT, H * 66], BF16, tag=f"v{b}", name=f"v_{b}")
        out["v"] = vsb
        nc.vector.memset(vsb[:, :, :].rearrange("p nt (h e) -> p nt h e", e=66)[:, :, :, 64], 1.0)
        for mo in range(NT):
            mr = P if mo < NT - 1 else 66  # tail rows incl one phantom (zero) row
            vv = vsb[:, mo].rearrange("p (h e) -> p h e", e=66)
            for (c0, cw) in CCH:
                pv = ps1.tile([P, 2, 512], F32, tag="ps1", name=f"pv_{b}_{mo}_{c0}")
                for ko in range(KO):
                    nc.tensor.matmul(pv[:mr, 0, :cw],
                                     lhsT=xT[:, ko, mo * P:mo * P + mr],
                                     rhs=wv_sb[:, ko, c0:c0 + cw],
                                     start=(ko == 0), stop=(ko == KO - 1))
                nc.vector.tensor_copy(
                    vv[:mr, c0 // 64:(c0 + cw) // 64, 0:64],
                    pv[:mr, 0, :cw].rearrange("p (h e) -> p h e", e=64))
            yield
        if dbg is not None and b == 0:
            nc.sync.dma_start(dbg["v"], vsb[:])

    # persistent P^T tiles (2 per instance); tail-pad rows zeroed once, exp
    # rewrites only rows :je each pair so the pads stay zero.
    pT4 = [pT_p.tile([P, NT, NPAD], BF16, tag=f"pT{i}", name=f"pT_{i}")
           for i in range(4)]
    for i in range(4):
        nc.vector.memset(pT4[i][64:66, NT - 1, :], 0.0)
    pT_of = {0: pT4[0:2], 1: pT4[2:4]}

    def gen_A(b, qkT, vsb, alpha_bc, out):
        """attention -> aT [128, KO, NPAD] bf16; yields after each head pair.

        Software-pipelined: the S^T+exp of pair p is interleaved (per j-tile)
        with the PV accumulation of pair p-1 so the PE streams matmuls while
        ACT runs the exps of the next pair.
        """
        aT = aT_pool.tile([P, KO, NPAD], BF16, tag=f"aT{b}", name=f"aT_{b}")
        out["aT"] = aT

        def emit_S_jo(hp, jo, tiles):
            heads = (2 * hp, 2 * hp + 1)
            je = _rows(jo)
            sp_pair = [ps2.tile([P, 2, 512], F32, tag="ps2", name=f"sp_{b}_{hp}_{jo}_{i}")
                       for i in range(2)]
            for ci, i0 in enumerate(ICH):
                for hi, h in enumerate(heads):
                    moK, moQ, pp = KO + h // 2, h // 2, (h % 2) * 64
                    nc.tensor.matmul(
                        sp_pair[hi][:je, ci, :IW],
                        lhsT=qkT[pp:pp + 64, moK, jo * P:jo * P + je],
                        rhs=qkT[pp:pp + 64, moQ, i0:i0 + IW],
                        start=True, stop=True)
            for hi in range(2):
                # exp(alpha * s); logits are small so no max subtraction.
                # pad row 65 of the tail tile stays zero (pre-zeroed).
                nc.scalar.activation(
                    tiles[hi][:je, jo].rearrange("p (c w) -> p c w", w=IW),
                    sp_pair[hi][:je, :, :IW], AF.Exp, scale=alpha_bc[:je])

        def emit_PV_head(h, hi, tiles, po):
            for ci, i0 in enumerate(ICH):
                for jo in range(NT):
                    jh = _rows(jo) if jo < NT - 1 else 66
                    nc.tensor.matmul(
                        po[:65, ci, :IW],
                        lhsT=vsb[:jh, jo, h * 66:h * 66 + 65],
                        rhs=tiles[hi][:jh, jo, i0:i0 + IW],
                        start=(jo == 0), stop=(jo == NT - 1))

        def emit_norm(h, po):
            if True:
                rden = rd_p.tile([P, NPAD], F32, tag="rd", name=f"rden_{b}_{h}")
                for ci, i0 in enumerate(ICH):
                    nc.vector.reciprocal(rden[64:65, i0:i0 + IW], po[64:65, ci, :IW])
                # partition_broadcast needs partition 0: DMA-shift row 64 -> 0
                rden0 = rd_p.tile([1, NPAD], F32, tag="rd0", name=f"rden0_{b}_{h}")
                nc.sync.dma_start(rden0[0:1, :], rden[64:65, :])
                rb = rb_p.tile([P, NPAD], F32, tag="rb", name=f"rb_{b}_{h}")
                nc.gpsimd.partition_broadcast(rb[:64], rden0[0:1, :])
                mo6 = h // 2
                if h % 2 == 0:
                    for ci, i0 in enumerate(ICH):
                        nc.vector.tensor_mul(aT[0:64, mo6, i0:i0 + IW],
                                             po[0:64, ci, :IW], rb[0:64, i0:i0 + IW])
                else:
                    tshift = tmp_p.tile([64, NPAD], BF16, tag="tmp", name=f"tsh_{b}_{h}")
                    for ci, i0 in enumerate(ICH):
                        nc.vector.tensor_mul(tshift[0:64, i0:i0 + IW],
                                             po[0:64, ci, :IW], rb[0:64, i0:i0 + IW])
                    nc.sync.dma_start(aT[64:128, mo6, :], tshift[0:64, :])

        for hp in range(H // 2):
            tiles = pT_of[b]
            for jo in range(NT):
                emit_S_jo(hp, jo, tiles)
            for hi, h in enumerate((2 * hp, 2 * hp + 1)):
                po = ps1.tile([P, 2, 512], F32, tag="ps1", name=f"po_{b}_{h}")
                emit_PV_head(h, hi, tiles, po)
                emit_norm(h, po)
            yield
        if dbg is not None and b == 0:
            nc.sync.dma_start(dbg["aT"], aT[:])

    def gen_P(b, aT):
        """proj + bias -> y; yields after each n-tile."""
        for mo in range(NT):
            rows = _rows(mo)
            ysb = y_p.tile([P, C], F32, tag="y", name=f"y_{b}_{mo}")
            for (c0, cw) in CCH:
                pp_ = ps1.tile([P, 2, 512], F32, tag="ps1", name=f"yp_{b}_{mo}_{c0}")
                for ko in range(KO):
                    nc.tensor.matmul(pp_[:rows, 0, :cw],
                                     lhsT=aT[:, ko, mo * P:mo * P + rows],
                                     rhs=projw_sb[:, ko, c0:c0 + cw],
                                     start=(ko == 0), stop=(ko == KO - 1))
                nc.vector.tensor_add(ysb[:rows, c0:c0 + cw], pp_[:rows, 0, :cw],
                                     pb_bc[:rows, c0:c0 + cw])
            nc.sync.dma_start(y_d[b, mo * P:mo * P + rows, :], ysb[:rows])
            yield

    def drain(g):
        for _ in g:
            pass

    # ---- schedule ----
    st0, st1 = {}, {}
    gQK0 = gen_QK(0, xTs[0], st0)
    drain(gQK0)
    a0 = phase_M(0, xTs[0])
    drain(gen_V(0, xTs[0], st0))

    # instance 1 qk/v/mlp interleave with instance 0 attention as PE filler
    # while ACT runs instance-0 exps.
    gA0 = gen_A(0, st0["qkT"], st0["v"], a0, st0)
    gQK1 = gen_QK(1, xTs[1], st1)
    gV1 = gen_V(1, xTs[1], st1)
    a1 = None
    step = 0
    while next(gA0, "end") != "end":
        for _ in range(2):
            if next(gQK1, "end") == "end" and a1 is None:
                a1 = phase_M(1, xTs[1])
        next(gV1, None)
        step += 1
    drain(gQK1)
    if a1 is None:
        a1 = phase_M(1, xTs[1])
    drain(gV1)

    gA1 = gen_A(1, st1["qkT"], st1["v"], a1, st1)
    gP0 = gen_P(0, st0["aT"])
    next(gP0, None)  # prime: fills A1's pair-0 exp window
    while next(gA1, "end") != "end":
        next(gP0, None)
    drain(gP0)
    drain(gen_P(1, st1["aT"]))


def build(debug=False):
    """Build and compile the per-core Bass module. Returns nc."""
    nc = bacc.Bacc("TRN2", target_bir_lowering=False, debug=False,
                   enable_asserts=False, num_devices=NCORES)
    io = {}
    io["x"] = nc.dram_tensor("x", [BPC, NXP, C], BF16, kind="ExternalInput").ap()
    io["qkv_w"] = nc.dram_tensor("qkv_w", [C, 3 * C], BF16, kind="ExternalInput").ap()
    io["proj_w"] = nc.dram_tensor("proj_w", [C, C], BF16, kind="ExternalInput").ap()
    io["proj_b"] = nc.dram_tensor("proj_b", [C], F32, kind="ExternalInput").ap()
    io["t_w1"] = nc.dram_tensor("t_w1", [C, HID], BF16, kind="ExternalInput").ap()
    io["t_b1"] = nc.dram_tensor("t_b1", [HID], F32, kind="ExternalInput").ap()
    io["t_w2"] = nc.dram_tensor("t_w2", [1, HID], BF16, kind="ExternalInput").ap()
    io["t_b2"] = nc.dram_tensor("t_b2", [1], F32, kind="ExternalInput").ap()
    io["y"] = nc.dram_tensor("y", [BPC, N, C], F32, kind="ExternalOutput").ap()

    dbg = None
    if debug:
        dbg = {
            "alpha": nc.dram_tensor("dbg_alpha", [P, 1], F32, kind="ExternalOutput").ap(),
            "xT": nc.dram_tensor("dbg_xT", [P, KO, NPAD], BF16, kind="ExternalOutput").ap(),
            "qkT": nc.dram_tensor("dbg_qkT", [P, 2 * KO, NPAD], BF16, kind="ExternalOutput").ap(),
            "v": nc.dram_tensor("dbg_v", [P, NT, H * 66], BF16, kind="ExternalOutput").ap(),
            "aT": nc.dram_tensor("dbg_aT", [P, KO, NPAD], BF16, kind="ExternalOutput").ap(),
        }
    with tile.TileContext(nc) as tc:
        _emit(tc, io, dbg)
    nc.compile()
    return nc


_NC_CACHE = None


def _get_nc():
    global _NC_CACHE
    if _NC_CACHE is None:
        _NC_CACHE = build()
    return _NC_CACHE


def make_in_maps(inputs: dict) -> list[dict]:
    bf = ml_dtypes.bfloat16
    x = np.asarray(inputs["x"], dtype=np.float32)
    xp = np.zeros((B, NXP, C), dtype=bf)
    xp[:, :N] = x.astype(bf)
    ws = {
        "qkv_w": np.ascontiguousarray(np.asarray(inputs["qkv_w"], np.float32).astype(bf)),
        "proj_w": np.ascontiguousarray(np.asarray(inputs["proj_w"], np.float32).astype(bf)),
        "proj_b": np.ascontiguousarray(np.asarray(inputs["proj_b"], np.float32)),
        "t_w1": np.ascontiguousarray(np.asarray(inputs["t_w1"], np.float32).astype(bf)),
        "t_b1": np.ascontiguousarray(np.asarray(inputs["t_b1"], np.float32)),
        "t_w2": np.ascontiguousarray(
            np.asarray(inputs["t_w2"], np.float32).reshape(1, HID).astype(bf)),
        "t_b2": np.ascontiguousarray(np.asarray(inputs["t_b2"], np.float32).reshape(1)),
    }
    return [dict(ws, x=np.ascontiguousarray(xp[i * BPC:(i + 1) * BPC]))
            for i in range(NCORES)]


def kernel(**inputs) -> np.ndarray:
    from concourse.bass_utils import run_bass_kernel_spmd
    nc = _get_nc()
    in_maps = make_in_maps(inputs)
    res = run_bass_kernel_spmd(nc, in_maps, core_ids=list(range(NCORES)))
    return np.concatenate([r["y"] for r in res.results], axis=0)


if __name__ == "__main__":
    rng = np.random.default_rng(0)
    ins = {
        "x": rng.standard_normal((B, N, C), dtype=np.float32),
        "qkv_w": (rng.standard_normal((C, 3 * C)) * 0.02).astype(np.float32),
        "proj_w": (rng.standard_normal((C, C)) * 0.02).astype(np.float32),
        "proj_b": np.zeros(C, np.float32),
        "t_w1": (rng.standard_normal((C, HID)) * 0.02).astype(np.float32),
        "t_b1": np.zeros(HID, np.float32),
        "t_w2": (rng.standard_normal((HID, 1)) * 0.02).astype(np.float32),
        "t_b2": np.zeros(1, np.float32),
    }
    out = kernel(**ins)
    print("out", out.shape, out.dtype, float(np.abs(out).max()))

